# revision 1
# baseline (speedup 1.0000x reference)
"""DDiT block kernel for 8 Trainium2 NeuronCores — v2 (fused phases).

Sharding: core c = (b = c//2, half = c%2).  Each core computes the block
output for its (batch, sequence-half): 1024 rows of 2048.  K/V are computed
redundantly for the full sequence on each core.  No collectives.

v2 changes vs v1:
- LN1 and QKV fused per s-tile (PE never idles waiting for LN1).
- Batched 3D-out DMA transposes (one instr per 1024-col tile, 8x fewer).
- 1024-wide bf16 matmuls / exps everywhere (psum tiles span 2 banks).
- Attention: scores and attn@V interleaved per k-tile (software pipeline),
  denominator broadcast moved from a f32 matmul to gpsimd.partition_broadcast,
  head-B normalize writes partitions 64:128 directly (no SBUF-shift DMA).
- Residual/gate elementwise work moved to the (otherwise idle) gpsimd engine.
- LN modulate ops run in bf16 (DVE 2x/4x modes).
- W_out/W2 prefetched so fc2 never stalls on weight DMA; w2 is pre-scaled
  by gate_mlp on host so fc2's epilogue is one add.
"""

import os
from contextlib import ExitStack

import numpy as np

B, S, D, H = 4, 2048, 1024, 16
HD = D // H  # 64
J = 4 * D  # 4096
SO = S // 2  # 1024 rows per core
SF = S  # full sequence
P = 128
EPS = 1e-6
N_CORES = 8

ST_F = SF // P  # 16 s-tiles full seq
ST_O = SO // P  # 8 s-tiles own half
DC = D // P  # 8 d-tiles
JT = J // P  # 32 j-tiles
HP = H // 2  # 8 head pairs


def _emit(ctx, nc, tc, io):
    import concourse.bass as bass
    import concourse.mybir as mybir

    f32 = mybir.dt.float32
    bf16 = mybir.dt.bfloat16
    AF = mybir.ActivationFunctionType
    OP = mybir.AluOpType

    def big(shape, dtype, name, side="left"):
        t, free = tc.tile(shape, dtype, name=name, side=side)
        return t, free

    def bcast_row(pool, key, n=D, dt=f32):
        """DMA a [n] dram row into a [P, n] sbuf tile, replicated across partitions."""
        ap_1d = io[key].ap()
        t = pool.tile([P, n], dt, tag=f"row_{key}", name=f"row_{key}", bufs=1)
        src = bass.AP(
            tensor=ap_1d.tensor,
            offset=ap_1d.offset,
            ap=[[0, P], list(ap_1d.ap[0])],
        )
        nc.sync.dma_start(out=t, in_=src)
        return t

    NB = 512  # psum bank width (f32 cols); matmuls may not cross banks

    def mmw(ps, lhsT, rhs, start, stop):
        for o in range(0, rhs.shape[-1], NB):
            nc.tensor.matmul(
                ps[:, o : o + NB], lhsT, rhs[..., o : o + NB],
                start=start, stop=stop,
            )

    eps_t, _free_eps = tc.tile([P, 1], f32, name="eps_t")
    nc.vector.memset(eps_t, EPS)

    def layernorm_mod(pool, x_ap, out_bf, a_bf, c_bf):
        # out_bf16 = ((x - mean) * rstd) * A + C   (A, C bf16 rows)
        stats = pool.tile([P, 2, 6], f32, tag="bnstats", name="stats")
        mv = pool.tile([P, 2], f32, tag="bnaggr", name="mv")
        xv = x_ap.rearrange("p (g k) -> p g k", g=2)
        for g in range(2):
            nc.vector.bn_stats(out=stats[:, g, :], in_=xv[:, g, :])
        nc.vector.bn_aggr(out=mv, in_=stats)
        rstd = pool.tile([P, 1], f32, tag="rstd", name="rstd")
        nc.scalar.activation(out=rstd, in_=mv[:, 1:2], func=AF.Sqrt, bias=eps_t)
        nc.vector.reciprocal(out=rstd, in_=rstd)
        tmp = pool.tile([P, D], bf16, tag="lntmp", name="lntmp")
        nc.vector.tensor_scalar(
            out=tmp, in0=x_ap, scalar1=mv[:, 0:1], scalar2=rstd,
            op0=OP.subtract, op1=OP.mult,
        )
        tmp2 = pool.tile([P, D], bf16, tag="lntmp2", name="lntmp2")
        nc.vector.tensor_mul(tmp2, tmp, a_bf)
        nc.vector.tensor_add(out_bf, tmp2, c_bf)

    # ======== persistent SBUF tensors (phase A) ========
    wqkv_sb, free_wqkv = big([P, DC, 3 * D], bf16, "wqkv_sb")  # 48KB/p
    yT, free_yT = big([P, DC, 2 * P], bf16, "yT")  # rolling 2-tile buffer
    qT, free_qT = big([P, HP, SO], bf16, "qT", side="right")
    kT, free_kT = big([P, HP, SF], bf16, "kT", side="right")
    v_aug, free_vaug = big([P, ST_F, H, 65], bf16, "v_aug", side="right")

    # ================ phase A: LN1 + QKV + rope, fused per s-tile ========
    with tc.tile_pool(name="pa", bufs=3) as pa, \
         tc.tile_pool(name="pas", bufs=4) as pas, \
         tc.tile_pool(name="psa", bufs=2, space="PSUM") as psa:
        nc.sync.dma_start(
            wqkv_sb, io["wqkv"].ap().rearrange("(dc p) c -> p dc c", p=P)
        )
        a1_t = bcast_row(pa, "a1", dt=bf16)
        c1_t = bcast_row(pa, "c1", dt=bf16)
        cos_t = pa.tile([P, ST_F, HD], bf16, tag="cos", name="cos_t", bufs=1)
        sin_t = pa.tile([P, ST_F, HD], bf16, tag="sin", name="sin_t", bufs=1)
        nc.sync.dma_start(cos_t, io["cos"].ap().rearrange("(t p) d -> p t d", p=P))
        nc.sync.dma_start(sin_t, io["sin"].ap().rearrange("(t p) d -> p t d", p=P))
        nc.vector.memset(v_aug[:, :, :, 64:65], 1.0)

        def rope(out_bf, qn, st):
            """Rotary on bf16 [P, 1024] (16 head-halves of 64)."""
            qv = qn.rearrange("p (h d) -> p h d", h=16)
            cos_b = cos_t[:, st, None, :].to_broadcast((P, 16, HD))
            sin_b = sin_t[:, st, None, :].to_broadcast((P, 16, HD))
            t1 = pas.tile([P, 16, HD], bf16, tag="ropet1", name="t1")
            nc.vector.tensor_mul(t1, qv, cos_b)
            qshuf = qn.rearrange("p (h two j) -> p h two j", h=16, two=2)[:, :, ::-1, :]
            t2 = pas.tile([P, 16, HD], bf16, tag="ropet2", name="t2")
            nc.vector.tensor_mul(
                t2.rearrange("p h (two j) -> p h two j", two=2),
                qshuf,
                sin_b.rearrange("p h (two j) -> p h two j", two=2),
            )
            nc.vector.tensor_add(out_bf.rearrange("p (h d) -> p h d", h=16), t1, t2)

        for st in range(ST_F):
            src = io["x_own"] if st < ST_O else io["x_oth"]
            row0 = (st % ST_O) * P
            x_t = pa.tile([P, D], f32, tag="xin", name="x_t")
            nc.sync.dma_start(x_t, src.ap()[row0 : row0 + P, :])
            y_t = pa.tile([P, D], bf16, tag="y_nat", name="y_t")
            layernorm_mod(pas, x_t, y_t, a1_t, c1_t)
            sl = (st % 2) * P
            nc.sync.dma_start(
                out=yT[:, :, sl : sl + P], in_=y_t, transpose=True
            )

            blocks = []
            if st < ST_O:
                blocks.append(("q", 0))
            blocks.append(("k", D))
            blocks.append(("v", 2 * D))
            for kind, c0 in blocks:
                ps = psa.tile([P, D], f32, tag="qkv_ps", name="qkv_ps")
                for dc in range(DC):
                    mmw(
                        ps,
                        yT[:, dc, sl : sl + P],
                        wqkv_sb[:, dc, c0 : c0 + D],
                        start=(dc == 0),
                        stop=(dc == DC - 1),
                    )
                if kind == "v":
                    nc.scalar.copy(
                        out=v_aug[:, st, :, 0:64],
                        in_=ps.rearrange("p (h d) -> p h d", h=16),
                    )
                else:
                    qn = pa.tile([P, D], bf16, tag="qn", name="qn")
                    nc.scalar.copy(out=qn, in_=ps)
                    rot = pa.tile([P, D], bf16, tag="rot", name="rot")
                    rope(rot, qn, st)
                    dst = qT if kind == "q" else kT
                    nc.sync.dma_start(
                        out=dst[:, :, st * P : (st + 1) * P], in_=rot,
                        transpose=True,
                    )
    free_yT()
    free_wqkv()

    # ======== persistent SBUF tensors (phase B+) ========
    w2sb, free_w2 = big([P, JT, D], bf16, "w2sb")  # 64KB/p (in freed wqkv/yT space)
    attnT, free_attnT = big([P, DC, SO], bf16, "attnT")
    wout_sb, free_wout = big([P, DC, D], bf16, "wout_sb")

    # ================ phase B: attention ================
    with tc.tile_pool(name="pb", bufs=3) as pb, \
         tc.tile_pool(name="pbs", bufs=1) as pbs, \
         tc.tile_pool(name="ps_sc", bufs=1, space="PSUM") as ps_sc, \
         tc.tile_pool(name="ps_u", bufs=1, space="PSUM") as ps_u:
        # prefetch wout + w2 (pre-scaled by gate_mlp on host) during attention
        nc.sync.dma_start(
            wout_sb, io["wout"].ap().rearrange("(dc p) c -> p dc c", p=P)
        )
        nc.sync.dma_start(w2sb, io["w2"].ap().rearrange("(o p) c -> p o c", p=P))
        ones_t = pb.tile([P, 64], f32, tag="ones", name="ones_t", bufs=1)
        nc.vector.memset(ones_t, 1.0)

        for hp in range(HP):
            probsA = [None] * ST_F
            probsB = [None] * ST_F
            psUA = ps_u.tile([P, SO], f32, tag="attA", name="psUA")
            psUB = ps_u.tile([P, SO], f32, tag="attB", name="psUB")

            def scores(kt):
                psA = ps_sc.tile([P, SO], f32, tag="scoreA", name="psA")
                psB = ps_sc.tile([P, SO], f32, tag="scoreB", name="psB")
                mmw(psA, kT[0:HD, hp, kt * P : (kt + 1) * P], qT[0:HD, hp, :],
                    start=True, stop=True)
                mmw(psB, kT[HD:P, hp, kt * P : (kt + 1) * P], qT[HD:P, hp, :],
                    start=True, stop=True)
                pA = pb.tile([P, SO], bf16, tag="probs", name="probsA", bufs=6)
                pB = pb.tile([P, SO], bf16, tag="probs", name="probsB", bufs=6)
                nc.scalar.activation(out=pA, in_=psA, func=AF.Exp, scale=0.125)
                nc.scalar.activation(out=pB, in_=psB, func=AF.Exp, scale=0.125)
                probsA[kt] = pA
                probsB[kt] = pB

            def attnv(kt):
                mmw(psUA[0:65], v_aug[:, kt, 2 * hp, :], probsA[kt],
                    start=(kt == 0), stop=(kt == ST_F - 1))
                mmw(psUB[0:65], v_aug[:, kt, 2 * hp + 1, :], probsB[kt],
                    start=(kt == 0), stop=(kt == ST_F - 1))

            scores(0)
            for kt in range(1, ST_F):
                scores(kt)
                attnv(kt - 1)
            attnv(ST_F - 1)

            # normalization: 1/denominator broadcast across 64 partitions
            recA = pbs.tile([P, SO], f32, tag="recA", name="recA")
            recB = pbs.tile([P, SO], f32, tag="recB", name="recB")
            nc.vector.reciprocal(recA[64:65, :], psUA[64:65, :])
            nc.vector.reciprocal(recB[64:65, :], psUB[64:65, :])
            bcA = ps_sc.tile([P, SO], f32, tag="scoreA", name="bcA")
            bcB = ps_sc.tile([P, SO], f32, tag="scoreB", name="bcB")
            mmw(bcA[0:64], ones_t[64:65, :], recA[64:65, :], start=True, stop=True)
            mmw(bcB[0:64], ones_t[64:65, :], recB[64:65, :], start=True, stop=True)
            bcsA = pbs.tile([P, SO], f32, tag="bcsA", name="bcsA")
            bcsB = pbs.tile([P, SO], f32, tag="bcsB", name="bcsB")
            nc.vector.tensor_copy(bcsA[0:64], bcA[0:64])
            nc.vector.tensor_copy(bcsB[0:64], bcB[0:64])
            nc.vector.tensor_mul(attnT[0:64, hp, :], psUA[0:64, :], bcsA[0:64])
            nc.vector.tensor_mul(attnT[64:128, hp, :], psUB[0:64, :], bcsB[0:64])
    free_vaug()
    free_kT()
    free_qT()

    # ======== phase C: out-proj + residual + LN2, fused per s-tile ========
    x_mid, free_xmid = big([P, ST_O, D], f32, "x_mid", side="right")
    y2T, free_y2T = big([P, DC, SO], bf16, "y2T", side="right")

    with tc.tile_pool(name="pc", bufs=3) as pc, \
         tc.tile_pool(name="pcs", bufs=4) as pcs, \
         tc.tile_pool(name="psc", bufs=2, space="PSUM") as psc:
        g1_t = bcast_row(pc, "g1")
        a2_t = bcast_row(pc, "a2", dt=bf16)
        c2_t = bcast_row(pc, "c2", dt=bf16)
        for st in range(ST_O):
            x_t = pc.tile([P, D], f32, tag="xin4", name="x_t4")
            nc.sync.dma_start(x_t, io["x_own"].ap()[st * P : (st + 1) * P, :])
            ps = psc.tile([P, D], f32, tag="outproj", name="op_ps")
            for dc in range(DC):
                mmw(
                    ps,
                    attnT[:, dc, st * P : (st + 1) * P],
                    wout_sb[:, dc, :],
                    start=(dc == 0),
                    stop=(dc == DC - 1),
                )
            t = pc.tile([P, D], f32, tag="op_t", name="op_t")
            nc.vector.tensor_mul(t, ps, g1_t)
            nc.gpsimd.tensor_add(x_mid[:, st, :], t, x_t)
            y2 = pc.tile([P, D], bf16, tag="y2", name="y2")
            layernorm_mod(pcs, x_mid[:, st, :], y2, a2_t, c2_t)
            nc.sync.dma_start(
                out=y2T[:, :, st * P : (st + 1) * P], in_=y2, transpose=True
            )
    free_wout()
    free_attnT()

    # ================ phase D: fc1 + gelu ================
    hT, free_hT = big([P, JT, SO], bf16, "hT")
    with tc.tile_pool(name="pd", bufs=3) as pd, \
         tc.tile_pool(name="psd", bufs=2, space="PSUM") as psd:
        fb1_t = pd.tile([P, JT], f32, tag="fb1", name="fb1_t", bufs=1)
        nc.sync.dma_start(fb1_t, io["fb1"].ap().rearrange("(o p) -> p o", p=P))
        w1_r = io["w1"].ap().rearrange("(dc p) c -> p dc c", p=P)
        for jt in range(JT):
            w1_t = pd.tile([P, DC, P], bf16, tag="w1", name="w1_t")
            nc.sync.dma_start(w1_t, w1_r[:, :, jt * P : (jt + 1) * P])
            ps = psd.tile([P, SO], f32, tag="fc1", name="fc1_ps")
            for dc in range(DC):
                mmw(
                    ps,
                    w1_t[:, dc, :],
                    y2T[:, dc, :],
                    start=(dc == 0),
                    stop=(dc == DC - 1),
                )
            hdst = hT[:, jt, :]
            if os.environ.get("KERNEL_SIM_GELU"):
                # sim lacks Gelu tables: tanh-approx gelu from primitives
                u = pd.tile([P, SO], f32, tag="gelu_u", name="gelu_u")
                nc.vector.tensor_scalar_add(u, ps, fb1_t[:, jt : jt + 1])
                t = pd.tile([P, SO], f32, tag="gelu_t", name="gelu_t")
                nc.vector.tensor_mul(t, u, u)
                nc.vector.tensor_mul(t, t, u)
                nc.vector.scalar_tensor_tensor(
                    out=t, in0=t, scalar=0.044715, in1=u,
                    op0=OP.mult, op1=OP.add,
                )
                nc.scalar.activation(
                    out=t, in_=t, func=AF.Tanh, scale=0.7978845608028654
                )
                nc.vector.tensor_scalar(
                    out=t, in0=t, scalar1=1.0, scalar2=0.5,
                    op0=OP.add, op1=OP.mult,
                )
                nc.vector.tensor_mul(hdst, u, t)
            else:
                nc.scalar.activation(
                    out=hdst,
                    in_=ps,
                    func=AF.Gelu_apprx_tanh,
                    bias=fb1_t[:, jt : jt + 1],
                )
    free_y2T()

    # ======== phase E: fc2 (w2 pre-scaled by gate) + final residual ========
    with tc.tile_pool(name="pe", bufs=3) as pe, \
         tc.tile_pool(name="pse", bufs=2, space="PSUM") as pse:
        gb2_t = bcast_row(pe, "gb2")
        for st in range(ST_O):
            ps = pse.tile([P, D], f32, tag="fc2", name="fc2_ps")
            for jt in range(JT):
                mmw(
                    ps,
                    hT[:, jt, st * P : (st + 1) * P],
                    w2sb[:, jt, :],
                    start=(jt == 0),
                    stop=(jt == JT - 1),
                )
            t = pe.tile([P, D], f32, tag="fin_t", name="fin_t")
            nc.vector.scalar_tensor_tensor(
                out=t, in0=ps, scalar=1.0, in1=x_mid[:, st, :],
                op0=OP.mult, op1=OP.add,
            )
            o_t = pe.tile([P, D], f32, tag="out", name="o_t")
            nc.gpsimd.tensor_add(o_t, t, gb2_t)
            nc.sync.dma_start(io["out"].ap()[st * P : (st + 1) * P, :], o_t)
    free_hT()
    free_w2()
    free_xmid()


def build_nc():
    import concourse.tile as tile
    import concourse.mybir as mybir
    from concourse import bacc

    f32 = mybir.dt.float32
    bf16 = mybir.dt.bfloat16

    nc = bacc.Bacc("TRN2", target_bir_lowering=False, debug=False)
    io = {}
    io["x_own"] = nc.dram_tensor("x_own", [SO, D], f32, kind="ExternalInput")
    io["x_oth"] = nc.dram_tensor("x_oth", [SO, D], f32, kind="ExternalInput")
    io["cos"] = nc.dram_tensor("cos", [SF, HD], bf16, kind="ExternalInput")
    io["sin"] = nc.dram_tensor("sin", [SF, HD], bf16, kind="ExternalInput")
    io["wqkv"] = nc.dram_tensor("wqkv", [D, 3 * D], bf16, kind="ExternalInput")
    io["wout"] = nc.dram_tensor("wout", [D, D], bf16, kind="ExternalInput")
    io["w1"] = nc.dram_tensor("w1", [D, J], bf16, kind="ExternalInput")
    io["w2"] = nc.dram_tensor("w2", [J, D], bf16, kind="ExternalInput")
    for name in ["a1", "c1", "a2", "c2"]:
        io[name] = nc.dram_tensor(name, [D], bf16, kind="ExternalInput")
    for name in ["g1", "gb2"]:
        io[name] = nc.dram_tensor(name, [D], f32, kind="ExternalInput")
    io["fb1"] = nc.dram_tensor("fb1", [J], f32, kind="ExternalInput")
    io["out"] = nc.dram_tensor("out", [SO, D], f32, kind="ExternalOutput")

    with tile.TileContext(nc) as tc:
        with ExitStack() as ctx:
            _emit(ctx, nc, tc, io)
    nc.finalize()
    return nc


def host_prep(inputs):
    """Build the 8 per-core input maps from the full problem inputs."""
    import ml_dtypes

    bf = ml_dtypes.bfloat16
    x = np.asarray(inputs["x"], np.float32)
    sigma_emb = np.asarray(inputs["sigma_emb"], np.float32)
    ada = sigma_emb @ np.asarray(inputs["ada_W"], np.float32) + np.asarray(
        inputs["ada_b"], np.float32
    )
    ada = ada.reshape(B, 6, D)
    shift_msa, scale_msa, gate_msa, shift_mlp, scale_mlp, gate_mlp = (
        ada[:, i] for i in range(6)
    )
    ln1_s = np.asarray(inputs["ln1_scale"], np.float32)
    ln1_b = np.asarray(inputs["ln1_bias"], np.float32)
    ln2_s = np.asarray(inputs["ln2_scale"], np.float32)
    ln2_b = np.asarray(inputs["ln2_bias"], np.float32)

    a1 = (ln1_s[None] * (1.0 + scale_msa)).astype(bf)  # [B, D]
    c1 = (ln1_b[None] * (1.0 + scale_msa) + shift_msa).astype(bf)
    a2 = (ln2_s[None] * (1.0 + scale_mlp)).astype(bf)
    c2 = (ln2_b[None] * (1.0 + scale_mlp) + shift_mlp).astype(bf)
    gb2 = gate_mlp * np.asarray(inputs["fc2_b"], np.float32)[None]

    # rope tables (match reference)
    inv_freq = 1.0 / (10000.0 ** (np.arange(0, HD, 2, dtype=np.float32) / HD))
    t = np.arange(S, dtype=np.float32)
    freqs = np.einsum("n,d->nd", t, inv_freq)
    emb = np.concatenate([freqs, freqs], axis=-1)  # [S, HD]
    cos = np.cos(emb).astype(bf)
    sin_signed = np.sin(emb).astype(np.float32)
    sin_signed[:, : HD // 2] *= -1.0  # fold rotate_half sign
    sin_signed = sin_signed.astype(bf)

    wqkv = np.asarray(inputs["W_qkv"], np.float32).astype(bf)
    wout = np.asarray(inputs["W_out"], np.float32).astype(bf)
    w1 = np.asarray(inputs["fc1_W"], np.float32).astype(bf)
    w2f = np.asarray(inputs["fc2_W"], np.float32)
    fb1 = np.asarray(inputs["fc1_b"], np.float32)

    in_maps = []
    for c in range(N_CORES):
        b, h = c // 2, c % 2
        own = slice(h * SO, (h + 1) * SO)
        oth = slice((1 - h) * SO, (2 - h) * SO)
        in_maps.append(
            {
                "x_own": np.ascontiguousarray(x[b, own]),
                "x_oth": np.ascontiguousarray(x[b, oth]),
                "cos": np.ascontiguousarray(np.concatenate([cos[own], cos[oth]], 0)),
                "sin": np.ascontiguousarray(
                    np.concatenate([sin_signed[own], sin_signed[oth]], 0)
                ),
                "wqkv": wqkv,
                "wout": wout,
                "w1": w1,
                "w2": (w2f * gate_mlp[b][None, :]).astype(bf),
                "a1": np.ascontiguousarray(a1[b]),
                "c1": np.ascontiguousarray(c1[b]),
                "g1": np.ascontiguousarray(gate_msa[b]),
                "a2": np.ascontiguousarray(a2[b]),
                "c2": np.ascontiguousarray(c2[b]),
                "gb2": np.ascontiguousarray(gb2[b]),
                "fb1": fb1,
            }
        )
    return in_maps


_NC_CACHE = {}


def kernel(**inputs) -> np.ndarray:
    import sys

    if "/opt/trn_rl_repo" not in sys.path:
        sys.path.insert(0, "/opt/trn_rl_repo")
    from concourse.bass_utils import run_bass_kernel_spmd

    in_maps = host_prep(inputs)
    if "nc" not in _NC_CACHE:
        _NC_CACHE["nc"] = build_nc()
    nc = _NC_CACHE["nc"]
    res = run_bass_kernel_spmd(
        nc,
        in_maps,
        core_ids=list(range(N_CORES)),
        trace=bool(int(os.environ.get("KERNEL_TRACE", "0"))),
    )
    out = np.empty((B, S, D), np.float32)
    for c in range(N_CORES):
        b, h = c // 2, c % 2
        out[b, h * SO : (h + 1) * SO] = res.results[c]["out"]
    _NC_CACHE["last_result"] = res
    return out



# revision 7
# speedup vs baseline: 2.9027x; 2.9027x over previous
"""DDiT block kernel for 8 Trainium2 NeuronCores — v3 (pipelined attention).

Sharding: core c = (b = c//2, half = c%2).  Each core computes the block
output for its (batch, sequence-half): 1024 rows of 2048.  K/V are computed
redundantly for the full sequence on each core.  No collectives.

v3 changes vs v2 (trace-driven):
- Attention rebuilt as a q-chunked (512) software pipeline with
  double-buffered score PSUM: ScalarE streams one Exp per k-tile
  ([128,1024] over both heads of a pair) back-to-back while the PE runs
  2 k-tiles ahead on scores and 1 behind on attn@V.  v2 serialised
  PE->exp->PE per k-tile (bufs=1 score psum), which also kept the PE
  HAM-throttled at half clock for the whole phase (~620us -> target ~300).
- wqkv DMA split per d-tile so the first QKV matmul starts after ~1/8 of
  the weight load; weight streams (wqkv/wout/w2/w1) moved to the second
  HWDGE queue (ScalarE-issued) so they never queue behind x loads and
  SBUF transposes on the sync queue.
- fc1 weights re-laid-out on host to [jt, p, dc*128] so each per-jt DMA
  reads 2KB contiguous per partition (v2's layout produced 256B packets
  at ~74GB/s, gating fc1).
"""

import os
from contextlib import ExitStack

import numpy as np

B, S, D, H = 4, 2048, 1024, 16
HD = D // H  # 64
J = 4 * D  # 4096
SO = S // 2  # 1024 rows per core
SF = S  # full sequence
P = 128
EPS = 1e-6
N_CORES = 8

ST_F = SF // P  # 16 s-tiles full seq
ST_O = SO // P  # 8 s-tiles own half
DC = D // P  # 8 d-tiles
JT = J // P  # 32 j-tiles
HP = H // 2  # 8 head pairs


def _emit(ctx, nc, tc, io):
    import concourse.bass as bass
    import concourse.mybir as mybir

    f32 = mybir.dt.float32
    bf16 = mybir.dt.bfloat16
    AF = mybir.ActivationFunctionType
    OP = mybir.AluOpType

    def big(shape, dtype, name, side="left"):
        t, free = tc.tile(shape, dtype, name=name, side=side)
        return t, free

    def bcast_row(pool, key, n=D, dt=f32):
        """DMA a [n] dram row into a [P, n] sbuf tile, replicated across partitions."""
        ap_1d = io[key].ap()
        t = pool.tile([P, n], dt, tag=f"row_{key}", name=f"row_{key}", bufs=1)
        src = bass.AP(
            tensor=ap_1d.tensor,
            offset=ap_1d.offset,
            ap=[[0, P], list(ap_1d.ap[0])],
        )
        nc.sync.dma_start(out=t, in_=src)
        return t

    NB = 512  # psum bank width (f32 cols); matmuls may not cross banks

    def mmw(ps, lhsT, rhs, start, stop):
        for o in range(0, rhs.shape[-1], NB):
            nc.tensor.matmul(
                ps[:, o : o + NB], lhsT, rhs[..., o : o + NB],
                start=start, stop=stop,
            )

    eps_t, _free_eps = tc.tile([P, 1], f32, name="eps_t")
    nc.vector.memset(eps_t, EPS)

    def layernorm_mod(pool, x_ap, out_bf, a_bf, c_bf):
        # out_bf16 = ((x - mean) * rstd) * A + C   (A, C bf16 rows)
        stats = pool.tile([P, 2, 6], f32, tag="bnstats", name="stats")
        mv = pool.tile([P, 2], f32, tag="bnaggr", name="mv")
        xv = x_ap.rearrange("p (g k) -> p g k", g=2)
        for g in range(2):
            nc.vector.bn_stats(out=stats[:, g, :], in_=xv[:, g, :])
        nc.vector.bn_aggr(out=mv, in_=stats)
        rstd = pool.tile([P, 1], f32, tag="rstd", name="rstd")
        nc.scalar.activation(out=rstd, in_=mv[:, 1:2], func=AF.Sqrt, bias=eps_t)
        nc.vector.reciprocal(out=rstd, in_=rstd)
        tmp = pool.tile([P, D], bf16, tag="lntmp", name="lntmp")
        nc.vector.tensor_scalar(
            out=tmp, in0=x_ap, scalar1=mv[:, 0:1], scalar2=rstd,
            op0=OP.subtract, op1=OP.mult,
        )
        tmp2 = pool.tile([P, D], bf16, tag="lntmp2", name="lntmp2")
        nc.vector.tensor_mul(tmp2, tmp, a_bf)
        nc.vector.tensor_add(out_bf, tmp2, c_bf)

    # ======== persistent SBUF tensors (phase A) ========
    wqkv_sb, free_wqkv = big([P, DC, 3 * D], bf16, "wqkv_sb")  # 48KB/p
    yT, free_yT = big([P, DC, 2 * P], bf16, "yT")  # rolling 2-tile buffer
    qT, free_qT = big([P, HP, SO], bf16, "qT", side="right")
    kT, free_kT = big([P, HP, SF], bf16, "kT", side="right")
    v_aug, free_vaug = big([P, ST_F, H, 65], bf16, "v_aug", side="right")

    # ================ phase A: LN1 + QKV + rope, fused per s-tile ========
    with tc.tile_pool(name="pa", bufs=3) as pa, \
         tc.tile_pool(name="pas", bufs=4) as pas, \
         tc.tile_pool(name="psa", bufs=2, space="PSUM") as psa:
        wqkv_r = io["wqkv"].ap().rearrange("(dc p) c -> p dc c", p=P)
        for dc in range(DC):
            nc.scalar.dma_start(wqkv_sb[:, dc, :], wqkv_r[:, dc, :])
        a1_t = bcast_row(pa, "a1", dt=bf16)
        c1_t = bcast_row(pa, "c1", dt=bf16)
        cos_t = pa.tile([P, ST_F, HD], bf16, tag="cos", name="cos_t", bufs=1)
        sin_t = pa.tile([P, ST_F, HD], bf16, tag="sin", name="sin_t", bufs=1)
        nc.sync.dma_start(cos_t, io["cos"].ap().rearrange("(t p) d -> p t d", p=P))
        nc.sync.dma_start(sin_t, io["sin"].ap().rearrange("(t p) d -> p t d", p=P))
        nc.vector.memset(v_aug[:, :, :, 64:65], 1.0)

        def rope(out_bf, qn, st):
            """Rotary on bf16 [P, 1024] (16 head-halves of 64)."""
            qv = qn.rearrange("p (h d) -> p h d", h=16)
            cos_b = cos_t[:, st, None, :].to_broadcast((P, 16, HD))
            sin_b = sin_t[:, st, None, :].to_broadcast((P, 16, HD))
            t1 = pas.tile([P, 16, HD], bf16, tag="ropet1", name="t1")
            nc.vector.tensor_mul(t1, qv, cos_b)
            qshuf = qn.rearrange("p (h two j) -> p h two j", h=16, two=2)[:, :, ::-1, :]
            t2 = pas.tile([P, 16, HD], bf16, tag="ropet2", name="t2")
            nc.vector.tensor_mul(
                t2.rearrange("p h (two j) -> p h two j", two=2),
                qshuf,
                sin_b.rearrange("p h (two j) -> p h two j", two=2),
            )
            nc.vector.tensor_add(out_bf.rearrange("p (h d) -> p h d", h=16), t1, t2)

        for st in range(ST_F):
            src = io["x_own"] if st < ST_O else io["x_oth"]
            row0 = (st % ST_O) * P
            x_t = pa.tile([P, D], f32, tag="xin", name="x_t")
            nc.sync.dma_start(x_t, src.ap()[row0 : row0 + P, :])
            y_t = pa.tile([P, D], bf16, tag="y_nat", name="y_t")
            layernorm_mod(pas, x_t, y_t, a1_t, c1_t)
            sl = (st % 2) * P
            nc.sync.dma_start(
                out=yT[:, :, sl : sl + P], in_=y_t, transpose=True
            )

            blocks = []
            if st < ST_O:
                blocks.append(("q", 0))
            blocks.append(("k", D))
            blocks.append(("v", 2 * D))
            for kind, c0 in blocks:
                ps = psa.tile([P, D], f32, tag="qkv_ps", name="qkv_ps")
                for dc in range(DC):
                    mmw(
                        ps,
                        yT[:, dc, sl : sl + P],
                        wqkv_sb[:, dc, c0 : c0 + D],
                        start=(dc == 0),
                        stop=(dc == DC - 1),
                    )
                if kind == "v":
                    nc.scalar.copy(
                        out=v_aug[:, st, :, 0:64],
                        in_=ps.rearrange("p (h d) -> p h d", h=16),
                    )
                else:
                    qn = pa.tile([P, D], bf16, tag="qn", name="qn")
                    nc.scalar.copy(out=qn, in_=ps)
                    rot = pa.tile([P, D], bf16, tag="rot", name="rot")
                    rope(rot, qn, st)
                    dst = qT if kind == "q" else kT
                    nc.sync.dma_start(
                        out=dst[:, :, st * P : (st + 1) * P], in_=rot,
                        transpose=True,
                    )
    free_yT()
    free_wqkv()

    # ======== persistent SBUF tensors (phase B+) ========
    w2sb, free_w2 = big([P, JT, D], bf16, "w2sb")  # 64KB/p (in freed wqkv/yT space)
    attnT, free_attnT = big([P, DC, SO], bf16, "attnT")
    wout_sb, free_wout = big([P, DC, D], bf16, "wout_sb")

    # ======== phase B: attention (q-chunked, ScalarE-saturated pipeline) ====
    QW = 512  # q-chunk width; scores for both heads of a pair share one exp
    QH = SO // QW
    with tc.tile_pool(name="pb", bufs=4) as pb, \
         tc.tile_pool(name="pbs", bufs=2) as pbs, \
         tc.tile_pool(name="ps_sc", bufs=2, space="PSUM") as ps_sc, \
         tc.tile_pool(name="ps_u", bufs=2, space="PSUM") as ps_u:
        # prefetch wout + w2 (pre-scaled by gate_mlp on host) during attention
        nc.scalar.dma_start(
            wout_sb, io["wout"].ap().rearrange("(dc p) c -> p dc c", p=P)
        )
        nc.scalar.dma_start(w2sb, io["w2"].ap().rearrange("(o p) c -> p o c", p=P))
        ones_t = pb.tile([P, 64], f32, tag="ones", name="ones_t", bufs=1)
        nc.vector.memset(ones_t, 1.0)

        for hp in range(HP):
            for qh in range(QH):
                q0 = qh * QW
                probs = [None] * ST_F
                psUA = ps_u.tile([P, QW], f32, tag="attA", name="psUA")
                psUB = ps_u.tile([P, QW], f32, tag="attB", name="psUB")

                def scores(kt):
                    # A in cols 0:QW (PE rows 0:64), B in cols QW:2QW (rows
                    # 64:128); the two MMs run concurrently in the array.
                    ps = ps_sc.tile([P, 2 * QW], f32, tag="scoreAB", name="psAB")
                    nc.tensor.matmul(
                        ps[:, 0:QW], kT[0:HD, hp, kt * P : (kt + 1) * P],
                        qT[0:HD, hp, q0 : q0 + QW], start=True, stop=True,
                    )
                    nc.tensor.matmul(
                        ps[:, QW : 2 * QW], kT[HD:P, hp, kt * P : (kt + 1) * P],
                        qT[HD:P, hp, q0 : q0 + QW], start=True, stop=True,
                    )
                    pr = pb.tile([P, 2 * QW], bf16, tag="probs", name="probs",
                                 bufs=4)
                    nc.scalar.activation(out=pr, in_=ps, func=AF.Exp, scale=0.125)
                    probs[kt] = pr

                def attnv(kt):
                    nc.tensor.matmul(
                        psUA[0:65, :], v_aug[:, kt, 2 * hp, :],
                        probs[kt][:, 0:QW],
                        start=(kt == 0), stop=(kt == ST_F - 1),
                    )
                    nc.tensor.matmul(
                        psUB[0:65, :], v_aug[:, kt, 2 * hp + 1, :],
                        probs[kt][:, QW : 2 * QW],
                        start=(kt == 0), stop=(kt == ST_F - 1),
                    )

                scores(0)
                scores(1)
                for kt in range(ST_F):
                    if kt + 2 < ST_F:
                        scores(kt + 2)
                    attnv(kt)

                # normalization: 1/denominator broadcast across 64 partitions
                rec = pbs.tile([P, 2 * QW], f32, tag="rec", name="rec")
                nc.vector.reciprocal(rec[64:65, 0:QW], psUA[64:65, :])
                nc.vector.reciprocal(rec[64:65, QW : 2 * QW], psUB[64:65, :])
                bc = ps_sc.tile([P, 2 * QW], f32, tag="scoreAB", name="bcAB")
                nc.tensor.matmul(bc[0:64, 0:QW], ones_t[64:65, :],
                                 rec[64:65, 0:QW], start=True, stop=True)
                nc.tensor.matmul(bc[0:64, QW : 2 * QW], ones_t[64:65, :],
                                 rec[64:65, QW : 2 * QW], start=True, stop=True)
                bcs = pbs.tile([P, 2 * QW], f32, tag="bcs", name="bcs")
                nc.vector.tensor_copy(bcs[0:64, :], bc[0:64, :])
                nc.vector.tensor_mul(attnT[0:64, hp, q0 : q0 + QW],
                                     psUA[0:64, :], bcs[0:64, 0:QW])
                nc.vector.tensor_mul(attnT[64:128, hp, q0 : q0 + QW],
                                     psUB[0:64, :], bcs[0:64, QW : 2 * QW])
    free_vaug()
    free_kT()
    free_qT()

    # ======== phase C: out-proj + residual + LN2, fused per s-tile ========
    x_mid, free_xmid = big([P, ST_O, D], f32, "x_mid", side="right")
    y2T, free_y2T = big([P, DC, SO], bf16, "y2T", side="right")

    with tc.tile_pool(name="pc", bufs=3) as pc, \
         tc.tile_pool(name="pcs", bufs=4) as pcs, \
         tc.tile_pool(name="psc", bufs=2, space="PSUM") as psc:
        g1_t = bcast_row(pc, "g1")
        a2_t = bcast_row(pc, "a2", dt=bf16)
        c2_t = bcast_row(pc, "c2", dt=bf16)
        for st in range(ST_O):
            x_t = pc.tile([P, D], f32, tag="xin4", name="x_t4")
            nc.sync.dma_start(x_t, io["x_own"].ap()[st * P : (st + 1) * P, :])
            ps = psc.tile([P, D], f32, tag="outproj", name="op_ps")
            for dc in range(DC):
                mmw(
                    ps,
                    attnT[:, dc, st * P : (st + 1) * P],
                    wout_sb[:, dc, :],
                    start=(dc == 0),
                    stop=(dc == DC - 1),
                )
            t = pc.tile([P, D], f32, tag="op_t", name="op_t")
            nc.vector.tensor_mul(t, ps, g1_t)
            nc.gpsimd.tensor_add(x_mid[:, st, :], t, x_t)
            y2 = pc.tile([P, D], bf16, tag="y2", name="y2")
            layernorm_mod(pcs, x_mid[:, st, :], y2, a2_t, c2_t)
            nc.sync.dma_start(
                out=y2T[:, :, st * P : (st + 1) * P], in_=y2, transpose=True
            )
    free_wout()
    free_attnT()

    # ================ phase D: fc1 + gelu ================
    hT, free_hT = big([P, JT, SO], bf16, "hT")
    with tc.tile_pool(name="pd", bufs=3) as pd, \
         tc.tile_pool(name="psd", bufs=2, space="PSUM") as psd:
        fb1_t = pd.tile([P, JT], f32, tag="fb1", name="fb1_t", bufs=1)
        nc.sync.dma_start(fb1_t, io["fb1"].ap().rearrange("(o p) -> p o", p=P))
        # w1 host-pre-arranged to [jt, p, dc*128]: 2KB contiguous per partition
        w1_r = io["w1"].ap().rearrange("(jt p) x -> jt p x", p=P)
        for jt in range(JT):
            w1_t = pd.tile([P, DC, P], bf16, tag="w1", name="w1_t")
            nc.scalar.dma_start(
                w1_t, w1_r[jt].rearrange("p (dc c) -> p dc c", dc=DC)
            )
            ps = psd.tile([P, SO], f32, tag="fc1", name="fc1_ps")
            for dc in range(DC):
                mmw(
                    ps,
                    w1_t[:, dc, :],
                    y2T[:, dc, :],
                    start=(dc == 0),
                    stop=(dc == DC - 1),
                )
            hdst = hT[:, jt, :]
            if os.environ.get("KERNEL_SIM_GELU"):
                # sim lacks Gelu tables: tanh-approx gelu from primitives
                u = pd.tile([P, SO], f32, tag="gelu_u", name="gelu_u")
                nc.vector.tensor_scalar_add(u, ps, fb1_t[:, jt : jt + 1])
                t = pd.tile([P, SO], f32, tag="gelu_t", name="gelu_t")
                nc.vector.tensor_mul(t, u, u)
                nc.vector.tensor_mul(t, t, u)
                nc.vector.scalar_tensor_tensor(
                    out=t, in0=t, scalar=0.044715, in1=u,
                    op0=OP.mult, op1=OP.add,
                )
                nc.scalar.activation(
                    out=t, in_=t, func=AF.Tanh, scale=0.7978845608028654
                )
                nc.vector.tensor_scalar(
                    out=t, in0=t, scalar1=1.0, scalar2=0.5,
                    op0=OP.add, op1=OP.mult,
                )
                nc.vector.tensor_mul(hdst, u, t)
            else:
                nc.scalar.activation(
                    out=hdst,
                    in_=ps,
                    func=AF.Gelu_apprx_tanh,
                    bias=fb1_t[:, jt : jt + 1],
                )
    free_y2T()

    # ======== phase E: fc2 (w2 pre-scaled by gate) + final residual ========
    with tc.tile_pool(name="pe", bufs=3) as pe, \
         tc.tile_pool(name="pse", bufs=2, space="PSUM") as pse:
        gb2_t = bcast_row(pe, "gb2")
        for st in range(ST_O):
            ps = pse.tile([P, D], f32, tag="fc2", name="fc2_ps")
            for jt in range(JT):
                mmw(
                    ps,
                    hT[:, jt, st * P : (st + 1) * P],
                    w2sb[:, jt, :],
                    start=(jt == 0),
                    stop=(jt == JT - 1),
                )
            t = pe.tile([P, D], f32, tag="fin_t", name="fin_t")
            nc.vector.scalar_tensor_tensor(
                out=t, in0=ps, scalar=1.0, in1=x_mid[:, st, :],
                op0=OP.mult, op1=OP.add,
            )
            o_t = pe.tile([P, D], f32, tag="out", name="o_t")
            nc.gpsimd.tensor_add(o_t, t, gb2_t)
            nc.sync.dma_start(io["out"].ap()[st * P : (st + 1) * P, :], o_t)
    free_hT()
    free_w2()
    free_xmid()


def build_nc():
    import concourse.tile as tile
    import concourse.mybir as mybir
    from concourse import bacc

    f32 = mybir.dt.float32
    bf16 = mybir.dt.bfloat16

    nc = bacc.Bacc("TRN2", target_bir_lowering=False, debug=False)
    io = {}
    io["x_own"] = nc.dram_tensor("x_own", [SO, D], f32, kind="ExternalInput")
    io["x_oth"] = nc.dram_tensor("x_oth", [SO, D], f32, kind="ExternalInput")
    io["cos"] = nc.dram_tensor("cos", [SF, HD], bf16, kind="ExternalInput")
    io["sin"] = nc.dram_tensor("sin", [SF, HD], bf16, kind="ExternalInput")
    io["wqkv"] = nc.dram_tensor("wqkv", [D, 3 * D], bf16, kind="ExternalInput")
    io["wout"] = nc.dram_tensor("wout", [D, D], bf16, kind="ExternalInput")
    io["w1"] = nc.dram_tensor("w1", [JT * P, DC * P], bf16, kind="ExternalInput")
    io["w2"] = nc.dram_tensor("w2", [J, D], bf16, kind="ExternalInput")
    for name in ["a1", "c1", "a2", "c2"]:
        io[name] = nc.dram_tensor(name, [D], bf16, kind="ExternalInput")
    for name in ["g1", "gb2"]:
        io[name] = nc.dram_tensor(name, [D], f32, kind="ExternalInput")
    io["fb1"] = nc.dram_tensor("fb1", [J], f32, kind="ExternalInput")
    io["out"] = nc.dram_tensor("out", [SO, D], f32, kind="ExternalOutput")

    with tile.TileContext(nc) as tc:
        with ExitStack() as ctx:
            _emit(ctx, nc, tc, io)
    nc.finalize()
    return nc


def host_prep(inputs):
    """Build the 8 per-core input maps from the full problem inputs."""
    import ml_dtypes

    bf = ml_dtypes.bfloat16
    x = np.asarray(inputs["x"], np.float32)
    sigma_emb = np.asarray(inputs["sigma_emb"], np.float32)
    ada = sigma_emb @ np.asarray(inputs["ada_W"], np.float32) + np.asarray(
        inputs["ada_b"], np.float32
    )
    ada = ada.reshape(B, 6, D)
    shift_msa, scale_msa, gate_msa, shift_mlp, scale_mlp, gate_mlp = (
        ada[:, i] for i in range(6)
    )
    ln1_s = np.asarray(inputs["ln1_scale"], np.float32)
    ln1_b = np.asarray(inputs["ln1_bias"], np.float32)
    ln2_s = np.asarray(inputs["ln2_scale"], np.float32)
    ln2_b = np.asarray(inputs["ln2_bias"], np.float32)

    a1 = (ln1_s[None] * (1.0 + scale_msa)).astype(bf)  # [B, D]
    c1 = (ln1_b[None] * (1.0 + scale_msa) + shift_msa).astype(bf)
    a2 = (ln2_s[None] * (1.0 + scale_mlp)).astype(bf)
    c2 = (ln2_b[None] * (1.0 + scale_mlp) + shift_mlp).astype(bf)
    gb2 = gate_mlp * np.asarray(inputs["fc2_b"], np.float32)[None]

    # rope tables (match reference)
    inv_freq = 1.0 / (10000.0 ** (np.arange(0, HD, 2, dtype=np.float32) / HD))
    t = np.arange(S, dtype=np.float32)
    freqs = np.einsum("n,d->nd", t, inv_freq)
    emb = np.concatenate([freqs, freqs], axis=-1)  # [S, HD]
    cos = np.cos(emb).astype(bf)
    sin_signed = np.sin(emb).astype(np.float32)
    sin_signed[:, : HD // 2] *= -1.0  # fold rotate_half sign
    sin_signed = sin_signed.astype(bf)

    wqkv = np.asarray(inputs["W_qkv"], np.float32).astype(bf)
    wout = np.asarray(inputs["W_out"], np.float32).astype(bf)
    # [D, J] -> [jt, p, dc, c] with w1[dc*128+p, jt*128+c] at [jt, p, dc, c]
    w1 = np.asarray(inputs["fc1_W"], np.float32).astype(bf)
    w1 = np.ascontiguousarray(
        w1.reshape(DC, P, JT, P).transpose(2, 1, 0, 3).reshape(JT * P, DC * P)
    )
    w2f = np.asarray(inputs["fc2_W"], np.float32)
    fb1 = np.asarray(inputs["fc1_b"], np.float32)

    in_maps = []
    for c in range(N_CORES):
        b, h = c // 2, c % 2
        own = slice(h * SO, (h + 1) * SO)
        oth = slice((1 - h) * SO, (2 - h) * SO)
        in_maps.append(
            {
                "x_own": np.ascontiguousarray(x[b, own]),
                "x_oth": np.ascontiguousarray(x[b, oth]),
                "cos": np.ascontiguousarray(np.concatenate([cos[own], cos[oth]], 0)),
                "sin": np.ascontiguousarray(
                    np.concatenate([sin_signed[own], sin_signed[oth]], 0)
                ),
                "wqkv": wqkv,
                "wout": wout,
                "w1": w1,
                "w2": (w2f * gate_mlp[b][None, :]).astype(bf),
                "a1": np.ascontiguousarray(a1[b]),
                "c1": np.ascontiguousarray(c1[b]),
                "g1": np.ascontiguousarray(gate_msa[b]),
                "a2": np.ascontiguousarray(a2[b]),
                "c2": np.ascontiguousarray(c2[b]),
                "gb2": np.ascontiguousarray(gb2[b]),
                "fb1": fb1,
            }
        )
    return in_maps


_NC_CACHE = {}


def kernel(**inputs) -> np.ndarray:
    import sys

    if "/opt/trn_rl_repo" not in sys.path:
        sys.path.insert(0, "/opt/trn_rl_repo")
    from concourse.bass_utils import run_bass_kernel_spmd

    in_maps = host_prep(inputs)
    if "nc" not in _NC_CACHE:
        _NC_CACHE["nc"] = build_nc()
    nc = _NC_CACHE["nc"]
    res = run_bass_kernel_spmd(
        nc,
        in_maps,
        core_ids=list(range(N_CORES)),
        trace=bool(int(os.environ.get("KERNEL_TRACE", "0"))),
    )
    out = np.empty((B, S, D), np.float32)
    for c in range(N_CORES):
        b, h = c // 2, c % 2
        out[b, h * SO : (h + 1) * SO] = res.results[c]["out"]
    _NC_CACHE["last_result"] = res
    return out

